# revision 1
# baseline (speedup 1.0000x reference)
"""Fused LayerNorm + multi-head attention + output projection on 8 TRN2 cores.

Reference computation (fp32):
    xn = LayerNorm(x) * gamma + beta
    q,k,v = split_heads(xn @ wq), ... ; scores = q k^T / sqrt(dh)
    out = softmax(scores) v ; out = out @ wo @ w_out + b_out

Sharding: batch*heads across 8 cores. Core c owns global heads {2c, 2c+1}
for both batches (inner columns [128c, 128c+128) of wq/wk/wv, same rows of
wo). wo and w_out are folded into one matrix host-side (both are static
weights), and gamma is folded into the qkv weights, so the device graph is:
    LN(no affine) -> transpose -> Q^T,K^T,V -> scores^T -> exp -> (P^T V and
    sum via ones-columns in one matmul) -> normalize -> woc partial matmul.
Each core emits a partial [1024, 4096] output (transposed layout); the host
sums the 8 partials, transposes, and adds b_out.

All matmul operands are bf16 (host-converted); PSUM accumulation stays fp32.
Working width is 1024 (tokens for stage A, queries for stage B); every
matmul is split into two bank-aligned 512-column chains (a matmul output
must stay inside one 2KB PSUM bank).  Measured end-to-end error of the bf16
pipeline is 4.2e-3 relative (gate: 2e-2).

Layout notes (everything chosen so the PE contracts along partitions and the
softmax denominator needs no cross-partition reduction):
  - xn^T [d=1024, n] built by PE-transposing LN output tiles (bf16, 1
    cycle/row, single-bank PSUM tiles).
  - K^T stored twice, zero-padded per head to the full 128 partitions, so
    the score matmuls are uniform 128-contract ops (the PE pstate ramps to
    2.4 GHz only on a uniform full-array stream; 64-contract quadrant
    matmuls kept it at 1.2 GHz).
  - V stored per k-tile as [128, 2, 128] where head 0 holds [V|ones] and
    head 1 [ones|V] so one PV matmul yields both the attention numerator
    and the softmax denominator (replicated across 64 partitions).
  - denominators: fast approximate reciprocal (single DVE op) — it
    miscompiles at partition base 64, so head 1 recips in place (base 0)
    and shifts up via DMA, head 0 copies out, shifts down, then recips.
Scheduling: three interlocking software pipelines keep every engine's
in-order queue from blocking a successor chunk's critical path:
  - LN stats run two chunks ahead (DVE), normalize one chunk ahead; the
    Q/K/V PSUM drains + V transposes of chunk c execute early in chunk c+1.
  - stage B runs scores(kt) / exp(kt) / PV(kt-2) software-pipelined, and
    the normalize + output projection of chunk c-1 executes inside chunk
    c's kt loop (one out-proj matmul + copy per iteration), so the PE
    never drains at chunk boundaries.
"""

import os
import sys

sys.path.insert(0, "/opt/trn_rl_repo")

import ml_dtypes
import numpy as np

import concourse.bass as bass
import concourse.bacc as bacc
import concourse.mybir as mybir
import concourse.tile as tile
from concourse.bass_utils import run_bass_kernel_spmd

B = 2
S = 2048
D = 1024
H = 16
DH = 64
N_TOK = B * S            # 4096
N_CORES = 8
HPC = 2                  # heads per core
ISL = HPC * DH           # per-core inner slice = 128
SCALE = DH ** -0.5
EPS = 1e-5

P = 128                  # partitions
NT = N_TOK // P          # 32 n-tiles
DK = D // P              # 8 d-tiles
CW = 512                 # stage-A chunk width (tokens)
NCH = CW // P            # 8 n-tiles per chunk
QW = 1024                # stage-B q-chunk width
NQC = S // QW            # 2 q-chunks per batch
KT = S // P              # 16 k-tiles per batch

f32 = mybir.dt.float32
bf16 = mybir.dt.bfloat16

DMA = "gpsimd"           # engine for dma_start


def _dma(nc):
    return getattr(nc, DMA)


def build_attention_core(has_bias=False):
    nc = bacc.Bacc("TRN2", target_bir_lowering=False, debug=False,
                   num_devices=N_CORES)
    x = nc.dram_tensor("x", [N_TOK, D], bf16, kind="ExternalInput").ap()
    wq = nc.dram_tensor("wq", [D, ISL], bf16, kind="ExternalInput").ap()
    wk = nc.dram_tensor("wk", [D, ISL], bf16, kind="ExternalInput").ap()
    wv = nc.dram_tensor("wv", [D, ISL], bf16, kind="ExternalInput").ap()
    woc = nc.dram_tensor("woc", [ISL, D], bf16, kind="ExternalInput").ap()
    bqkv = nc.dram_tensor("bqkv", [ISL, 3], f32, kind="ExternalInput").ap()
    ident = nc.dram_tensor("ident", [P, P], bf16, kind="ExternalInput").ap()
    vones = nc.dram_tensor("vones", [P, DH], bf16, kind="ExternalInput").ap()
    out_t = nc.dram_tensor("out_t", [D, N_TOK], f32, kind="ExternalOutput").ap()

    with tile.TileContext(nc) as tc:
        with tc.tile_pool(name="persist", bufs=1) as persist:
            qt_sb = persist.tile([P, N_TOK], bf16, tag="qt")
            # per-head zero-padded K^T copies: head hh occupies rows
            # hh*64..hh*64+64, the other rows stay zero.  Scores then run as
            # full 128-contract matmuls (the zero rows contribute nothing),
            # keeping the PE in a single uniform tile config.
            kt0_sb = persist.tile([P, N_TOK], bf16, tag="kt0")
            kt1_sb = persist.tile([P, N_TOK], bf16, tag="kt1")
            v_sb = persist.tile([P, NT, HPC, P], bf16, tag="v")
            id_sb = persist.tile([P, P], bf16, tag="ident")
            wq_sb = persist.tile([P, DK, ISL], bf16, tag="wq")
            wk_sb = persist.tile([P, DK, ISL], bf16, tag="wk")
            wv_sb = persist.tile([P, DK, ISL], bf16, tag="wv")
            woc_sb = persist.tile([P, D], bf16, tag="woc")
            bq_sb = persist.tile([P, 3], f32, tag="bqkv")
            eps_sb = persist.tile([P, 1], f32, tag="eps")

            dma = _dma(nc)
            dma.dma_start(id_sb[:], ident)
            # wq [D, ISL] -> [p, dk, ISL] with d = dk*128 + p
            dma.dma_start(wq_sb[:], wq.rearrange("(dk p) m -> p dk m", p=P))
            dma.dma_start(wk_sb[:], wk.rearrange("(dk p) m -> p dk m", p=P))
            dma.dma_start(wv_sb[:], wv.rearrange("(dk p) m -> p dk m", p=P))
            dma.dma_start(woc_sb[:], woc)
            dma.dma_start(bq_sb[:], bqkv)
            nc.vector.memset(eps_sb[:], EPS)
            nc.vector.memset(kt0_sb[DH:P, :], 0.0)
            nc.vector.memset(kt1_sb[0:DH, :], 0.0)
            # ones columns for the softmax-denominator trick, broadcast-DMA'd
            # from a DRAM constant (memset can't write bf16-into-matmul-ready
            # layout as cheaply)
            vones_b = bass.AP(tensor=vones.tensor, offset=0,
                              ap=[[DH, P], [0, NT], [1, DH]])
            dma.dma_start(v_sb[:, :, 0, DH:P], vones_b)
            dma.dma_start(v_sb[:, :, 1, 0:DH], vones_b)

            # ---------------- Stage A: LN -> xn^T -> Q^T/K^T/V ----------
            with tc.tile_pool(name="ln", bufs=20) as ln_pool, \
                 tc.tile_pool(name="xn", bufs=12) as xn_pool, \
                 tc.tile_pool(name="lnst", bufs=4) as st_pool, \
                 tc.tile_pool(name="xnt", bufs=2 * DK) as xnt_pool, \
                 tc.tile_pool(name="vtmp", bufs=2) as vtmp_pool, \
                 tc.tile_pool(name="ps_t", bufs=4, space="PSUM") as ps_t, \
                 tc.tile_pool(name="ps_acc", bufs=1, space="PSUM") as ps_acc:
                # LN is software-pipelined one chunk ahead: the DVE stats
                # stream for chunk c+1 is emitted between chunk c's
                # transposes and its QKV matmuls, so neither the DVE (stats)
                # nor the ACT (normalize) ever gates the PE at a chunk
                # boundary.  Normalize runs on ACT as
                #   xn = Copy(x * rstd + (-mu*rstd))
                # with per-partition (per-token) scale/bias APs.
                def emit_ln_stats(ch):
                    x_tiles = []
                    mv8 = st_pool.tile([P, NCH, 2], f32, tag="mv8")
                    for j in range(NCH):
                        nt = ch * NCH + j
                        x_t = ln_pool.tile([P, D], bf16, tag="x", name="x_t")
                        dma.dma_start(x_t[:], x[nt * P:(nt + 1) * P, :])
                        stats = st_pool.tile([P, 2, 6], f32, tag="stats",
                                             name="stats")
                        for g in range(2):
                            nc.vector.bn_stats(
                                out=stats[:, g, :],
                                in_=x_t[:, g * 512:(g + 1) * 512])
                        nc.vector.bn_aggr(out=mv8[:, j, :], in_=stats[:])
                        x_tiles.append(x_t)
                    return (x_tiles, mv8)

                def emit_ln_normalize(state):
                    x_tiles, mv8 = state
                    # batched rstd for the whole chunk: sqrt(var+eps), recip
                    rstd8 = st_pool.tile([P, NCH], f32, tag="rstd8")
                    nc.scalar.activation(
                        out=rstd8[:], in_=mv8[:, :, 1],
                        func=mybir.ActivationFunctionType.Sqrt,
                        bias=eps_sb[:], scale=1.0)
                    nc.vector.reciprocal(out=rstd8[:], in_=rstd8[:])
                    xn_tiles = []
                    for j in range(NCH):
                        xn_t = xn_pool.tile([P, D], bf16, tag="xn",
                                            name="xn_t")
                        nc.vector.tensor_scalar(
                            out=xn_t[:], in0=x_tiles[j][:],
                            scalar1=mv8[:, j, 0:1], scalar2=rstd8[:, j:j + 1],
                            op0=mybir.AluOpType.subtract,
                            op1=mybir.AluOpType.mult)
                        xn_tiles.append(xn_t)
                    return xn_tiles

                # LN stats run TWO chunks ahead: when chunk c's body emits
                # the normalize for chunk c+1, its stats (and hence the ACT
                # sqrt's input) are already computed, so neither ACT nor DVE
                # ever stalls the PE stream.
                # The Q/K/V PSUM copy-outs, the V transposes, and the V-block
                # copies of chunk c all depend on chunk c's PE tail; emitting
                # them early in chunk c+1's body keeps every engine's
                # next-chunk queue head unblocked.
                def emit_qkv_drain(fin):
                    ch_p, acc_q, acc_k, acc_v = fin
                    c_cols = slice(ch_p * CW, (ch_p + 1) * CW)
                    if has_bias:
                        nc.vector.tensor_scalar_add(
                            out=qt_sb[:, c_cols], in0=acc_q[:],
                            scalar1=bq_sb[:, 0:1])
                        nc.vector.tensor_scalar_add(
                            out=kt0_sb[0:DH, c_cols], in0=acc_k[0:DH, :],
                            scalar1=bq_sb[0:DH, 1:2])
                        nc.vector.tensor_scalar_add(
                            out=kt1_sb[DH:P, c_cols], in0=acc_k[DH:P, :],
                            scalar1=bq_sb[DH:P, 1:2])
                    else:
                        nc.scalar.copy(out=qt_sb[:, c_cols], in_=acc_q[:])
                        nc.scalar.copy(out=kt0_sb[0:DH, c_cols],
                                       in_=acc_k[0:DH, :])
                        nc.scalar.copy(out=kt1_sb[DH:P, c_cols],
                                       in_=acc_k[DH:P, :])
                    vt_tmp = vtmp_pool.tile([P, CW], bf16, tag="vt")
                    if has_bias:
                        nc.vector.tensor_scalar_add(
                            out=vt_tmp[:], in0=acc_v[:], scalar1=bq_sb[:, 2:3])
                    else:
                        nc.scalar.copy(out=vt_tmp[:], in_=acc_v[:])
                    return vt_tmp

                def emit_v_transposes(fin, vt_tmp):
                    ch_p = fin[0]
                    for j in range(NCH):
                        nt = ch_p * NCH + j
                        # shares the "tp" slots: PSUM is fully budgeted
                        # (tp 2 banks + q/k/v accumulators 6 banks)
                        tpv = ps_t.tile([P, P], bf16, tag="tp", name="tpv")
                        nc.tensor.transpose(
                            tpv[:], vt_tmp[:, j * P:(j + 1) * P], id_sb[:])
                        # head 0 block: [V | ones]; head 1 block: [ones | V]
                        # single strided copy: dst halves sit 192 elems
                        # apart in v_sb's free dim, src halves 64 apart
                        dst = bass.AP(
                            tensor=v_sb.tensor, offset=v_sb.offset + nt * 2 * P,
                            ap=[list(v_sb.ap[0]), [DH + P, 2], [1, DH]])
                        src = bass.AP(
                            tensor=tpv.tensor, offset=tpv.offset,
                            ap=[list(tpv.ap[0]), [DH, 2], [1, DH]])
                        nc.vector.tensor_copy(dst, src)

                NCHUNKS = N_TOK // CW
                ln_states = {0: emit_ln_stats(0)}
                xn_tiles = emit_ln_normalize(ln_states.pop(0))
                if NCHUNKS > 1:
                    ln_states[1] = emit_ln_stats(1)
                pend_acc = None
                for ch in range(NCHUNKS):
                    # Interleaved transpose + QKV accumulation, software-
                    # pipelined over dk: the transposes for dk+1 are issued
                    # before the three projection matmuls for dk, so the PE
                    # stream has no gap while the PSUM->SBUF copy of xnt[dk]
                    # completes.  Q/K/V accumulate simultaneously in three
                    # PSUM accumulators (6 banks; a matmul output must stay
                    # inside one PSUM bank = 512 fp32, so each is two
                    # bank-aligned 512-col chains).
                    acc_q = ps_acc.tile([P, CW], f32, tag="accq", name="acc_q")
                    acc_k = ps_acc.tile([P, CW], f32, tag="acck", name="acc_k")
                    acc_v = ps_acc.tile([P, CW], f32, tag="accv", name="acc_v")
                    accs = ((acc_q, wq_sb), (acc_k, wk_sb), (acc_v, wv_sb))
                    xnt_tiles = []

                    def do_transposes(dk):
                        tp = ps_t.tile([P, CW], bf16, tag="tp", name="tp")
                        for j in range(NCH):
                            nc.tensor.transpose(
                                tp[:, j * P:(j + 1) * P],
                                xn_tiles[j][:, dk * P:(dk + 1) * P],
                                id_sb[:])
                        xnt = xnt_pool.tile([P, CW], bf16, tag="xnt",
                                            name="xnt")
                        # all on ACT: the DVE stream carries next chunk's LN
                        # stats and must not block on PE transposes
                        nc.scalar.copy(xnt[:], tp[:])
                        xnt_tiles.append(xnt)

                    # Emission order per body: previous chunk's Q/K/V
                    # drain (ACT head, inputs ready), normalize for chunk
                    # c+1 (sqrt input ready), stats for chunk c+2, this
                    # chunk's transposes, previous chunk's V transposes +
                    # copies, then this chunk's QKV matmul chains.
                    vt_prev = emit_qkv_drain(pend_acc) if pend_acc else None
                    next_xn = (emit_ln_normalize(ln_states.pop(ch + 1))
                               if ch + 1 < NCHUNKS else None)
                    if ch + 2 < NCHUNKS:
                        ln_states[ch + 2] = emit_ln_stats(ch + 2)
                    for dk in range(DK):
                        do_transposes(dk)
                    if vt_prev is not None:
                        emit_v_transposes(pend_acc, vt_prev)
                    for dk in range(DK):
                        for (acc, w_sb) in accs:
                            for h in range(CW // 512):
                                c_sl = slice(h * 512, (h + 1) * 512)
                                nc.tensor.matmul(
                                    acc[:, c_sl], w_sb[:, dk, :],
                                    xnt_tiles[dk][:, c_sl],
                                    start=(dk == 0), stop=(dk == DK - 1))
                    pend_acc = (ch, acc_q, acc_k, acc_v)
                    xn_tiles = next_xn
                # drain the final chunk's Q/K/V
                vt_prev = emit_qkv_drain(pend_acc)
                emit_v_transposes(pend_acc, vt_prev)

            # ---------------- Stage B: attention + output proj ----------
            with tc.tile_pool(name="exp", bufs=6) as exp_pool, \
                 tc.tile_pool(name="attn", bufs=2) as attn_pool, \
                 tc.tile_pool(name="ot", bufs=2) as ot_pool, \
                 tc.tile_pool(name="so", bufs=3) as so_pool, \
                 tc.tile_pool(name="ps_s", bufs=2, space="PSUM") as ps_s, \
                 tc.tile_pool(name="ps_u", bufs=1, space="PSUM") as ps_u:
                # Cross-chunk software pipeline: the normalize + output-
                # projection of chunk c-1 is deferred into the kt loop of
                # chunk c, so the PE never drains at a chunk boundary.
                # The out-proj PSUM tiles reuse the ut slots of chunk c-1
                # (legal: they are consumed by the multiplies first), one
                # po per kt iteration with its PSUM->SBUF copy on DVE.
                #
                # Per chunk:
                #   [end of kt loop]  finish_pre: denominator reciprocals
                #       (fast approx works only at partition base 0, so
                #       hh=1 recips straight from PSUM and shifts up, hh=0
                #       copies out, shifts down, then recips)
                #   [kt==1 of next]   finish_mul: ut * (1/den) -> ot
                #   [kt>=1 of next]   one po matmul + DVE copy + DMA per kt
                pending_fin = None
                pending_pos = []

                def emit_finish_mul(fin):
                    uts_p, ot_p, dn0_p, dn1_p = (fin["uts"], fin["ot"],
                                                 fin["dn0"], fin["dn1"])
                    for h in range(2):
                        c_sl = slice(h * 512, (h + 1) * 512)
                        nc.vector.tensor_mul(
                            ot_p[0:DH, c_sl], uts_p[0][0:DH, c_sl],
                            dn0_p[0:DH, c_sl])
                        nc.vector.tensor_mul(
                            ot_p[DH:P, c_sl], uts_p[1][DH:P, c_sl],
                            dn1_p[DH:P, c_sl])

                def emit_finish_po(fin, m):
                    ot_p, q_sl_p = fin["ot"], fin["q_sl"]
                    po = ps_s.tile([P, QW], f32, tag="st", name="po")
                    for h in range(2):
                        c_sl = slice(h * 512, (h + 1) * 512)
                        nc.tensor.matmul(
                            po[:, c_sl], woc_sb[:, m * P:(m + 1) * P],
                            ot_p[:, c_sl], start=True, stop=True)
                    so = so_pool.tile([P, QW], f32, tag="so")
                    nc.vector.tensor_copy(so[:], po[:])
                    dma.dma_start(
                        out_t[m * P:(m + 1) * P, q_sl_p], so[:])

                chunks = [(b, qc) for b in range(B) for qc in range(NQC)]
                for (b, qc) in chunks:
                    qb = b * S
                    q_sl = slice(qb + qc * QW, qb + (qc + 1) * QW)
                    ot_t = ot_pool.tile([P, QW], bf16, tag="ot")
                    uts = [ps_u.tile([P, QW], f32, tag=f"ut{hh}",
                                     name=f"ut{hh}", bufs=1)
                           for hh in range(HPC)]
                    # PV runs two iterations behind the score matmuls so the
                    # deferred finish work of the previous chunk (multiplies
                    # on DVE at kt==1) completes before PV(kt=0) needs the
                    # freed ut banks.
                    es_hist = {}
                    for kt in range(KT + 2):
                        sts = []
                        if kt < KT:
                            k_sl = slice(qb + kt * P, qb + (kt + 1) * P)
                            sts = [ps_s.tile([P, QW], f32, tag="st",
                                             name="st")
                                   for _ in range(HPC)]
                            kts = (kt0_sb, kt1_sb)
                            for h in range(2):
                                c_sl = slice(h * 512, (h + 1) * 512)
                                qc_sl = slice(
                                    qb + qc * QW + h * 512,
                                    qb + qc * QW + (h + 1) * 512)
                                for hh in range(HPC):
                                    nc.tensor.matmul(
                                        sts[hh][:, c_sl],
                                        kts[hh][:, k_sl],
                                        qt_sb[:, qc_sl],
                                        start=True, stop=True)
                        if kt >= 2:
                            ktpv = kt - 2
                            vkt = b * KT + ktpv
                            es_pv = es_hist.pop(ktpv)
                            for h in range(2):
                                c_sl = slice(h * 512, (h + 1) * 512)
                                for hh in range(HPC):
                                    nc.tensor.matmul(
                                        uts[hh][:, c_sl],
                                        v_sb[:, vkt, hh, :],
                                        es_pv[hh][:, c_sl],
                                        start=(ktpv == 0),
                                        stop=(ktpv == KT - 1))
                        if kt < KT:
                            cur = []
                            for hh in range(HPC):
                                es = exp_pool.tile([P, QW], bf16,
                                                   tag="es", name="es")
                                nc.scalar.activation(
                                    out=es[:], in_=sts[hh][:],
                                    func=mybir.ActivationFunctionType.Exp,
                                    scale=SCALE)
                                cur.append(es)
                            es_hist[kt] = cur
                        # deferred finish work of the previous chunk
                        if pending_fin is not None:
                            if kt == 1:
                                emit_finish_mul(pending_fin)
                            if 2 <= kt <= DK + 1:
                                emit_finish_po(pending_fin, kt - 2)
                                if kt == DK + 1:
                                    pending_fin = None
                    # finish_pre for this chunk: reciprocals of the softmax
                    # denominators
                    dr1 = attn_pool.tile([P, QW], f32, tag="dr", name="dr1")
                    dn1 = attn_pool.tile([P, QW], f32, tag="dn", name="dn1")
                    nc.vector.reciprocal_approx_fast(
                        out=dr1[0:DH, :], in_=uts[1][0:DH, :])
                    dma.dma_start(dn1[DH:P, :], dr1[0:DH, :])
                    ds0 = attn_pool.tile([P, QW], f32, tag="dr", name="ds0")
                    dn0 = attn_pool.tile([P, QW], f32, tag="dn", name="dn0")
                    nc.scalar.copy(ds0[DH:P, :], uts[0][DH:P, :])
                    dma.dma_start(dn0[0:DH, :], ds0[DH:P, :])
                    nc.vector.reciprocal_approx_fast(
                        out=dn0[0:DH, :], in_=dn0[0:DH, :])
                    pending_fin = {"uts": uts, "ot": ot_t, "dn0": dn0,
                                   "dn1": dn1, "q_sl": q_sl}
                # drain the last chunk
                emit_finish_mul(pending_fin)
                for m in range(DK):
                    emit_finish_po(pending_fin, m)
    nc.compile()
    return nc


_NC_CACHE = {}
LAST_RESULTS = None


def _get_nc(has_bias=False):
    key = has_bias
    if key not in _NC_CACHE:
        _NC_CACHE[key] = build_attention_core(has_bias)
    return _NC_CACHE[key]


def kernel(x, gamma, beta, wq, wk, wv, wo, w_out, b_out):
    x = np.ascontiguousarray(np.asarray(x, dtype=np.float32)).reshape(N_TOK, D)
    gamma = np.asarray(gamma, dtype=np.float32)
    beta = np.asarray(beta, dtype=np.float32)
    wq = np.asarray(wq, dtype=np.float32)
    wk = np.asarray(wk, dtype=np.float32)
    wv = np.asarray(wv, dtype=np.float32)
    wo = np.asarray(wo, dtype=np.float32)
    w_out = np.asarray(w_out, dtype=np.float32)
    b_out = np.asarray(b_out, dtype=np.float32)

    # Host-side weight folding (all static weights):
    #   gamma folds into wq/wk/wv rows; beta contributes per-column biases;
    #   wo @ w_out collapses the two output projections.
    bf = ml_dtypes.bfloat16
    wq_g = (gamma[:, None] * wq).astype(bf)
    wk_g = (gamma[:, None] * wk).astype(bf)
    wv_g = (gamma[:, None] * wv).astype(bf)
    woc_full = (wo.astype(np.float64) @ w_out.astype(np.float64)).astype(bf)
    bq = beta @ wq
    bk = beta @ wk
    bv = beta @ wv

    x_bf = x.astype(bf)
    ident = np.eye(P, dtype=bf)
    in_maps = []
    for c in range(N_CORES):
        sl = slice(c * ISL, (c + 1) * ISL)
        in_maps.append({
            "x": x_bf,
            "wq": np.ascontiguousarray(wq_g[:, sl]),
            "wk": np.ascontiguousarray(wk_g[:, sl]),
            "wv": np.ascontiguousarray(wv_g[:, sl]),
            "woc": np.ascontiguousarray(woc_full[sl, :]),
            "bqkv": np.ascontiguousarray(
                np.stack([bq[sl], bk[sl], bv[sl]], axis=1).astype(np.float32)),
            "ident": ident,
            "vones": np.ones((P, DH), dtype=bf),
        })

    has_bias = bool(np.any(beta != 0.0))
    nc = _get_nc(has_bias)
    trace = os.environ.get("ATT_TRACE", "0") == "1"
    kwargs = {}
    if trace:
        tdir = os.environ.get("ATT_TRACE_DIR", "/tmp/att_trace")
        os.makedirs(tdir, exist_ok=True)
        kwargs = dict(trace=True, tmpdir=tdir)
    res = run_bass_kernel_spmd(nc, in_maps, core_ids=list(range(N_CORES)),
                               **kwargs)
    global LAST_RESULTS
    LAST_RESULTS = res
    acc = np.zeros((D, N_TOK), dtype=np.float32)
    for c in range(N_CORES):
        acc += res.results[c]["out_t"]
    out = acc.T.reshape(B, S, D) + b_out
    return out.astype(np.float32)


if __name__ == "__main__":
    rng = np.random.default_rng(0)
    inputs = {
        "x": rng.standard_normal((B, S, D), dtype=np.float32),
        "gamma": np.ones(D, np.float32),
        "beta": np.zeros(D, np.float32),
        "wq": rng.standard_normal((D, D), dtype=np.float32) * 0.02,
        "wk": rng.standard_normal((D, D), dtype=np.float32) * 0.02,
        "wv": rng.standard_normal((D, D), dtype=np.float32) * 0.02,
        "wo": rng.standard_normal((D, D), dtype=np.float32) * 0.02,
        "w_out": rng.standard_normal((D, D), dtype=np.float32) * 0.02,
        "b_out": np.zeros(D, np.float32),
    }
    out = kernel(**inputs)
    print("out", out.shape, out.dtype, float(np.abs(out).max()))

